# revision 46
# baseline (speedup 1.0000x reference)
"""Trainium2 Bass kernel for nn_CachedAttention (8-core SPMD, tensor-parallel heads).

Contract: kernel(**inputs) takes the FULL unsharded inputs from
reference.setup_inputs() and returns the FULL (1, 2048, 2048) f32 output.

Math notes (validated vs the reference in numpy/bf16 at ~5.8e-3 rel err):
- The reference applies a TOP-LEFT-aligned causal mask tril(T, S) over the
  concatenated [cache; new] sequence, so new token t only attends to
  positions 0..t — all inside the 2048-entry cache. The freshly projected
  k/v (wk, wv, k-norm, k-rope) are therefore completely masked out and
  never computed here.
- RMSNorm's per-token scale commutes with RoPE (both linear), and q_norm_w
  folds into transposed RoPE tables CT/ST indexed [d, t].
- Scores ~ N(0,1), so softmax runs without the max-subtraction pass.

Design (collective-free, measured 159.2us vs 184.4us a2a baseline):
- Head-sharded: core c owns q heads {2c, 2c+1} and kv head c.
- q-projection computed TRANSPOSED (qT[d, t] = wq_h @ xT) so scores need
  no PE transposes. rmsnorm rstd: square (DVE) -> all-ones matmul
  (reduce+broadcast across partitions in one 213ns MM) -> Sqrt (ACT) ->
  reciprocal_approx_fast (single DVE op).
- Scores land in f32 PSUM pairs [128, 1024] (2 banks) -> ONE exp per pair
  (halves ACT's ~352-cycle per-instruction overhead).
- attV computed TRANSPOSED (attT[d, t] = V^T @ P, lhsT = v tile as stored)
  with N=512 streams; softmax denominator: full tiles ride the PE as
  accumulating all-ones matmuls into PSUM, diagonal tiles accumulate on
  DVE in bf16 (noise averages out in the 128-partition reduction), then
  fast-reciprocal + normalize fused with the PSUM->SBUF evacuation.
- NO collective: each core emits a full [2048, 2048] bf16 PARTIAL output
  (its 2 heads through the matching wo rows); the host sums 8 partials.
  wo chains queue as filler closures drained one per scores/attv pair, so
  PE stays dense while ScalarE paces exp; attention groups run with g0
  inside the x-DMA-paced projection phase and the rest largest-first.
- Evacuations (PSUM f32 -> SBUF bf16) alternate DVE/ScalarE; DMA cannot
  read PSUM and GpSimd has no PSUM port on TRN2, so these two engines are
  the only path out (GpSimd handles the causal tri-masks instead).
"""

import math
import sys

import numpy as np

sys.path.insert(0, "/opt/trn_rl_repo")

import ml_dtypes

P = 128
T = 2048
DM = 2048
DK = 128
HLOC = 2          # q heads per core
NCORES = 8
ND = DM // P      # 16 contraction chunks
NT = T // P       # 16 token tiles
GW = 4            # token tiles per attention group (512 wide)
NG = NT // GW     # 4 groups
TCH = 512         # proj token chunk
EPS = 1e-6
ROPE_BASE = 10000.0

_bf16 = ml_dtypes.bfloat16


def _build_module():
    import concourse.tile as tile
    from concourse import bacc, mybir

    bf = mybir.dt.bfloat16
    f32 = mybir.dt.float32
    AF = mybir.ActivationFunctionType

    nc = bacc.Bacc("TRN2", target_bir_lowering=False, debug=False, num_devices=NCORES)

    xT = nc.dram_tensor("xT", [P, 4, ND, TCH], bf, kind="ExternalInput").ap()
    wqT = nc.dram_tensor("wqT", [P, HLOC, ND, P], bf, kind="ExternalInput").ap()
    kcT = nc.dram_tensor("kcT", [P, T], bf, kind="ExternalInput").ap()
    vp = nc.dram_tensor("vp", [P, NT, DK], bf, kind="ExternalInput").ap()
    woT = nc.dram_tensor("woT", [P, HLOC, DM], bf, kind="ExternalInput").ap()
    ct = nc.dram_tensor("ct", [P, T], bf, kind="ExternalInput").ap()
    st = nc.dram_tensor("st", [P, T], bf, kind="ExternalInput").ap()
    tri = nc.dram_tensor("tri", [P, P], bf, kind="ExternalInput").ap()
    out = nc.dram_tensor("out", [T, DM], bf, kind="ExternalOutput").ap()
    out_r = out.rearrange("(ti p) f -> p ti f", p=P)

    with tile.TileContext(nc) as tc:
        with (
            tc.tile_pool(name="res", bufs=1) as res,
            tc.tile_pool(name="work", bufs=6) as work,
            tc.tile_pool(name="probs", bufs=3) as probs,
            tc.tile_pool(name="accp", bufs=2) as accp,
            tc.tile_pool(name="small", bufs=4) as small,
            tc.tile_pool(name="bcast", bufs=2) as bcastp,
            tc.tile_pool(name="outp", bufs=3) as outp,
            tc.tile_pool(name="ps_sc", bufs=2, space="PSUM") as ps_sc,
            tc.tile_pool(name="ps_wo", bufs=2, space="PSUM") as ps_wo,
            tc.tile_pool(name="ps_gx", bufs=2, space="PSUM") as ps_gx,
        ):
            # ---- loads, in consumption order; x split in half-chunks so
            # the first projection matmuls start ~5.6us in ----
            wq_sb = res.tile([P, HLOC, ND, P], bf)
            nc.sync.dma_start(wq_sb[:, 0], wqT[:, 0])
            eps_sb = res.tile([P, 1], f32)
            nc.vector.memset(eps_sb, EPS)
            # all-ones stationary: ones128.T @ x = column sums broadcast to
            # every output partition — reduce+broadcast in one 213ns matmul
            ones_sb = res.tile([P, P], bf)
            nc.vector.memset(ones_sb, 1.0)
            x_sb = res.tile([P, 4, ND, TCH], bf)
            ct_sb = res.tile([P, T], bf)
            st_sb = res.tile([P, T], bf)
            kc_sb = res.tile([P, T], bf)
            vp_sb = res.tile([P, NT, DK], bf)
            tri_sb = res.tile([P, P], bf)
            wo_sb = res.tile([P, HLOC, DM], bf)
            cs0 = slice(0, TCH)
            nc.sync.dma_start(x_sb[:, 0, 0:4], xT[:, 0, 0:4])
            nc.sync.dma_start(x_sb[:, 0, 4:8], xT[:, 0, 4:8])
            nc.sync.dma_start(wq_sb[:, 1], wqT[:, 1])
            nc.sync.dma_start(x_sb[:, 0, 8:16], xT[:, 0, 8:16])
            nc.sync.dma_start(ct_sb[:, cs0], ct[:, cs0])
            nc.sync.dma_start(st_sb[:, cs0], st[:, cs0])
            nc.sync.dma_start(kc_sb, kcT)
            nc.sync.dma_start(vp_sb, vp)
            nc.sync.dma_start(tri_sb, tri)
            for tck in range(1, 4):
                csk = slice(tck * TCH, (tck + 1) * TCH)
                if tck == 3:
                    nc.sync.dma_start(wo_sb, woT)
                nc.sync.dma_start(x_sb[:, tck, 0:8], xT[:, tck, 0:8])
                nc.sync.dma_start(x_sb[:, tck, 8:16], xT[:, tck, 8:16])
                nc.sync.dma_start(ct_sb[:, csk], ct[:, csk])
                nc.sync.dma_start(st_sb[:, csk], st[:, csk])

            qr = [res.tile([P, T], bf, name=f"qr{h}") for h in range(HLOC)]
            qn = [res.tile([P, T], bf, name=f"qn{h}") for h in range(HLOC)]
            attT = [res.tile([P, T], bf, name=f"attT{h}") for h in range(HLOC)]

            # ---- phase B: transposed q projection + rope + rmsnorm ----
            def proj_chunk(tck, h):
                cs = slice(tck * TCH, (tck + 1) * TCH)
                ps = ps_wo.tile([P, TCH], f32, tag="pw")
                for dc in range(ND):
                    nc.tensor.matmul(
                        ps,
                        lhsT=wq_sb[:, h, dc, :],
                        rhs=x_sb[:, tck, dc, :],
                        start=(dc == 0),
                        stop=(dc == ND - 1),
                    )
                qsb = work.tile([P, TCH], bf, tag="qsb")
                nc.vector.tensor_copy(qsb, ps)
                # rope: qr = q*CT + swap64(q)*ST.  st_sb holds ST with
                # halves pre-swapped so each mul's two SB inputs share a
                # base partition (only the OUT is partition-shifted).
                u = work.tile([P, TCH], bf, tag="u")
                nc.vector.tensor_mul(
                    u[0:64, :], qsb[64:128, :], st_sb[64:128, cs])
                nc.vector.tensor_mul(
                    u[64:128, :], qsb[0:64, :], st_sb[0:64, cs])
                t1 = work.tile([P, TCH], bf, tag="t1")
                nc.vector.tensor_mul(t1, qsb, ct_sb[:, cs])
                nc.vector.tensor_add(qr[h][:, cs], t1, u)
                # rmsnorm: sumsq bcast via ones-matmul, then
                # rstd = 1/sqrt(ssq/DK + eps) via Sqrt (ACT) + fast recip
                sq = work.tile([P, TCH], bf, tag="sq")
                nc.vector.tensor_mul(sq, qsb, qsb)
                ssqb = ps_gx.tile([P, TCH], f32, tag="gx")
                nc.tensor.matmul(
                    ssqb, lhsT=ones_sb, rhs=sq, start=True, stop=True)
                # rstd = exp(-0.5*ln(ssq/DK + eps)): Ln and Exp share
                # one ACT table set, so no table thrash against attention
                lns = work.tile([P, TCH], bf, tag="lns")
                nc.scalar.activation(
                    out=lns, in_=ssqb, func=AF.Ln,
                    bias=eps_sb, scale=1.0 / DK)
                rstdb = work.tile([P, TCH], bf, tag="rstdb")
                nc.scalar.activation(
                    out=rstdb, in_=lns, func=AF.Exp, scale=-0.5)
                nc.vector.tensor_mul(qn[h][:, cs], qr[h][:, cs], rstdb)

            # ---- phase C pieces ----
            # wo chains queue up as closures; attention drains one per
            # scores/attv pair so PE always has independent filler work
            filler = []

            def emit_filler(n=1):
                for _ in range(n):
                    if filler:
                        filler.pop(0)()

            def att_group(g, h, fill=False):
                gs = slice(g * GW * P, (g + 1) * GW * P)
                po = ps_gx.tile([P, GW * P], f32, tag="gx")
                acc = accp.tile([P, GW * P], bf, tag="acc")
                dnb = ps_gx.tile([P, GW * P], f32, tag="gx")
                nsi = GW * (g + 1)
                pbs = []

                def scores_pair(pi):
                    ps2 = ps_sc.tile([P, 2 * GW * P], f32, tag="sc")
                    los = []
                    for j in range(2):
                        si = pi * 2 + j
                        k = si - GW * g
                        lo = k * P if k > 0 else 0
                        los.append(lo)
                        nc.tensor.matmul(
                            ps2[:, j * 512 + lo:(j + 1) * 512],
                            lhsT=kc_sb[:, si * P:(si + 1) * P],
                            rhs=qn[h][:, g * 512 + lo:(g + 1) * 512],
                            start=True, stop=True,
                        )
                    pb = probs.tile([P, 2 * GW * P], bf, tag="pb")
                    nc.scalar.activation(
                        out=pb[:, los[0]:], in_=ps2[:, los[0]:],
                        func=AF.Exp)
                    for j in range(2):
                        si = pi * 2 + j
                        k = si - GW * g
                        if k >= 0:
                            dsl = slice(j * 512 + k * P,
                                        j * 512 + (k + 1) * P)
                            nc.gpsimd.tensor_mul(
                                pb[:, dsl], pb[:, dsl], tri_sb)
                    pbs.append(pb)

                def attv_pair(pi):
                    pb = pbs[pi]
                    for j in range(2):
                        si = pi * 2 + j
                        k = si - GW * g
                        lo = k * P if k > 0 else 0
                        if k < 0:
                            # full tile: denominator row rides on PE
                            # (ones-matmul accumulate into dnb)
                            nc.tensor.matmul(
                                dnb, lhsT=ones_sb,
                                rhs=pb[:, j * 512:(j + 1) * 512],
                                start=(si == 0), stop=False,
                            )
                        else:
                            # diagonal tile: accumulate on DVE in bf16
                            # (noise averages out in the reduction)
                            if k == 0:
                                nc.vector.tensor_copy(
                                    acc, pb[:, j * 512:(j + 1) * 512])
                            else:
                                nc.vector.tensor_add(
                                    acc[:, lo:512], acc[:, lo:512],
                                    pb[:, j * 512 + lo:(j + 1) * 512])
                        nc.tensor.matmul(
                            po[:, lo:512],
                            lhsT=vp_sb[:, si, :],
                            rhs=pb[:, j * 512 + lo:(j + 1) * 512],
                            start=(si == 0), stop=(si == nsi - 1),
                        )

                npairs = 2 * (g + 1)
                for pi in range(npairs):
                    scores_pair(pi)
                    if fill:
                        emit_filler()
                    if pi >= 1:
                        attv_pair(pi - 1)
                        if fill:
                            emit_filler()
                attv_pair(npairs - 1)
                if fill:
                    emit_filler()
                # fold the diagonal accumulator into dnb, closing the
                # PE accumulation group
                nc.tensor.matmul(
                    dnb, lhsT=ones_sb, rhs=acc,
                    start=(g == 0), stop=True)

                # reciprocal -> normalize-evacuate
                rcb = bcastp.tile([P, GW * P], f32, tag="rcb")
                nc.vector.reciprocal_approx_fast(rcb, dnb)
                nc.vector.tensor_mul(attT[h][:, gs], po, rcb)

            def wo_chain(ti, fc, evac_eng, pool=None):
                fs = slice(fc * 512, (fc + 1) * 512)
                pw = (pool or ps_wo).tile(
                    [P, 512], f32, tag="pw" if pool is None else "gx")
                nc.tensor.matmul(
                    pw,
                    lhsT=attT[0][:, ti * P:(ti + 1) * P],
                    rhs=wo_sb[:, 0, fs],
                    start=True, stop=False,
                )
                nc.tensor.matmul(
                    pw,
                    lhsT=attT[1][:, ti * P:(ti + 1) * P],
                    rhs=wo_sb[:, 1, fs],
                    start=False, stop=True,
                )
                osb = outp.tile([P, 512], bf, tag="osb")
                if evac_eng == 0:
                    nc.vector.tensor_copy(osb, pw)
                else:
                    nc.scalar.copy(osb, pw)
                nc.sync.dma_start(out_r[:, ti, fs], osb)

            def push_wo(tis, dve_frac=2, pools=False):
                # dve_frac of 3 evacs to DVE, rest to Scalar; pools=True
                # alternates PSUM pools (tail: gx ring is free then)
                for i, (ti, fc) in enumerate(
                        [(ti, fc) for ti in tis for fc in range(4)]):
                    eng = 0 if (i % 3) < dve_frac else 1
                    pl = ps_gx if (pools and i % 2) else None
                    filler.append(
                        lambda ti=ti, fc=fc, e=eng, p=pl: wo_chain(
                            ti, fc, e, p))

            # ---- emission schedule ----
            # g0's attention interleaves into the projection phase (PE
            # filler while x chunks stream in); the remaining groups run
            # largest-first so PE stays dense and HAM stays warm; wo
            # chains drain as filler once wo_sb has landed (~att g2).
            proj_chunk(0, 0)
            proj_chunk(0, 1)
            att_group(0, 0)
            att_group(0, 1)
            for tck in range(1, 4):
                proj_chunk(tck, 0)
                proj_chunk(tck, 1)
            push_wo([0, 1, 2, 3], dve_frac=1)
            att_group(3, 0, fill=True)
            att_group(3, 1, fill=True)
            push_wo([12, 13, 14, 15], dve_frac=1)
            att_group(2, 0, fill=True)
            att_group(2, 1, fill=True)
            push_wo([8, 9, 10, 11], dve_frac=2)
            att_group(1, 0, fill=True)
            att_group(1, 1, fill=True)
            push_wo([4, 5, 6, 7], dve_frac=2)
            emit_filler(len(filler))

    nc.compile()
    return nc


def _host_inputs(x, cached_k, cached_v, wq, wo, q_norm_w):
    """Build the 8 per-core input maps (host-side shard + fold + cast)."""
    xt = np.ascontiguousarray(x[0].T).astype(np.float32)          # (DM, T)
    # x_prep[p, tck, dc, t'] = x[tck*512+t', dc*128+p]
    x_prep = np.ascontiguousarray(
        xt.reshape(ND, P, 4, TCH).transpose(1, 2, 0, 3)).astype(_bf16)

    inv_freq = 1.0 / (ROPE_BASE ** (np.arange(0, DK, 2, dtype=np.float32) / DK))
    ang = np.arange(T, dtype=np.float32)[:, None] * inv_freq[None, :]  # (T,64)
    w = q_norm_w.astype(np.float32)
    cosT = np.cos(ang).T                                           # (64, T)
    sinT = np.sin(ang).T
    CT = np.concatenate([cosT, cosT], axis=0) * w[:, None]         # (128, T)
    ST = np.empty((DK, T), np.float32)
    ST[:64] = -w[64:, None] * sinT
    ST[64:] = w[:64, None] * sinT
    CTb = np.ascontiguousarray(CT).astype(_bf16)
    # halves pre-swapped: st_sb[p] = ST[(p+64) % 128]
    STb = np.ascontiguousarray(
        np.concatenate([ST[64:], ST[:64]], axis=0)).astype(_bf16)

    tri_m = (np.arange(P)[:, None] <= np.arange(P)[None, :]).astype(_bf16)

    in_maps = []
    for c in range(NCORES):
        # wq_prep[p, h, dc, m] = wq[(2c+h)*128+m, dc*128+p]
        wqs = wq[c * HLOC * DK:(c + 1) * HLOC * DK, :].astype(np.float32)
        wq_prep = np.ascontiguousarray(
            wqs.reshape(HLOC, P, ND, P).transpose(3, 0, 2, 1)).astype(_bf16)
        kcp = np.ascontiguousarray(
            cached_k[c].T / math.sqrt(DK)).astype(_bf16)           # (128, T)
        vpp = np.ascontiguousarray(
            cached_v[c].reshape(NT, P, DK).transpose(1, 0, 2)).astype(_bf16)
        # wo_prep[p, h, f] = wo[f, (2c+h)*128+p]
        wos = wo[:, c * HLOC * DK:(c + 1) * HLOC * DK].astype(np.float32)
        wo_prep = np.ascontiguousarray(
            wos.reshape(DM, HLOC, P).transpose(2, 1, 0)).astype(_bf16)
        in_maps.append({
            "xT": x_prep, "wqT": wq_prep, "kcT": kcp, "vp": vpp,
            "woT": wo_prep, "ct": CTb, "st": STb, "tri": tri_m,
        })
    return in_maps


_CACHED = {}


def _get_module():
    if "nc" not in _CACHED:
        _CACHED["nc"] = _build_module()
    return _CACHED["nc"]


def run(inputs, trace=False, **kw):
    """Compile (cached), run on 8 cores, return (output, BassKernelResults)."""
    from concourse import bass_utils

    nc = _get_module()
    in_maps = _host_inputs(
        np.asarray(inputs["x"], np.float32),
        np.asarray(inputs["cached_k"], np.float32),
        np.asarray(inputs["cached_v"], np.float32),
        np.asarray(inputs["wq"], np.float32),
        np.asarray(inputs["wo"], np.float32),
        np.asarray(inputs["q_norm_w"], np.float32),
    )
    res = bass_utils.run_bass_kernel_spmd(
        nc, in_maps, core_ids=list(range(NCORES)), trace=trace, **kw)
    full = np.zeros((T, DM), np.float32)
    for c in range(NCORES):
        full += res.results[c]["out"].astype(np.float32)
    return full.reshape(1, T, DM), res


def kernel(**inputs):
    full, _ = run(inputs)
    return full


# revision 47
# speedup vs baseline: 1.0306x; 1.0306x over previous
"""Trainium2 Bass kernel for nn_CachedAttention (8-core SPMD, tensor-parallel heads).

Contract: kernel(**inputs) takes the FULL unsharded inputs from
reference.setup_inputs() and returns the FULL (1, 2048, 2048) f32 output.

Math notes (validated vs the reference in numpy/bf16 at ~5.8e-3 rel err):
- The reference applies a TOP-LEFT-aligned causal mask tril(T, S) over the
  concatenated [cache; new] sequence, so new token t only attends to
  positions 0..t — all inside the 2048-entry cache. The freshly projected
  k/v (wk, wv, k-norm, k-rope) are therefore completely masked out and
  never computed here.
- RMSNorm's per-token scale commutes with RoPE (both linear), and q_norm_w
  folds into transposed RoPE tables CT/ST indexed [d, t].
- Scores ~ N(0,1), so softmax runs without the max-subtraction pass.

Design (collective-free, measured 159.2us vs 184.4us a2a baseline):
- Head-sharded: core c owns q heads {2c, 2c+1} and kv head c.
- q-projection computed TRANSPOSED (qT[d, t] = wq_h @ xT) so scores need
  no PE transposes. rmsnorm rstd: square (DVE) -> all-ones matmul
  (reduce+broadcast across partitions in one 213ns MM) -> Sqrt (ACT) ->
  reciprocal_approx_fast (single DVE op).
- Scores land in f32 PSUM pairs [128, 1024] (2 banks) -> ONE exp per pair
  (halves ACT's ~352-cycle per-instruction overhead).
- attV computed TRANSPOSED (attT[d, t] = V^T @ P, lhsT = v tile as stored)
  with N=512 streams; softmax denominator: full tiles ride the PE as
  accumulating all-ones matmuls into PSUM, diagonal tiles accumulate on
  DVE in bf16 (noise averages out in the 128-partition reduction), then
  fast-reciprocal + normalize fused with the PSUM->SBUF evacuation.
- NO collective: each core emits a full [2048, 2048] bf16 PARTIAL output
  (its 2 heads through the matching wo rows); the host sums 8 partials.
  wo chains queue as filler closures drained one per scores/attv pair, so
  PE stays dense while ScalarE paces exp; attention groups run with g0
  inside the x-DMA-paced projection phase and the rest largest-first.
- Evacuations (PSUM f32 -> SBUF bf16) alternate DVE/ScalarE; DMA cannot
  read PSUM and GpSimd has no PSUM port on TRN2, so these two engines are
  the only path out (GpSimd handles the causal tri-masks instead).
"""

import math
import sys

import numpy as np

sys.path.insert(0, "/opt/trn_rl_repo")

import ml_dtypes

P = 128
T = 2048
DM = 2048
DK = 128
HLOC = 2          # q heads per core
NCORES = 8
ND = DM // P      # 16 contraction chunks
NT = T // P       # 16 token tiles
GW = 4            # token tiles per attention group (512 wide)
NG = NT // GW     # 4 groups
TCH = 512         # proj token chunk
EPS = 1e-6
ROPE_BASE = 10000.0

_bf16 = ml_dtypes.bfloat16


def _build_module():
    import concourse.tile as tile
    from concourse import bacc, mybir

    bf = mybir.dt.bfloat16
    f32 = mybir.dt.float32
    AF = mybir.ActivationFunctionType

    nc = bacc.Bacc("TRN2", target_bir_lowering=False, debug=False, num_devices=NCORES)

    xT = nc.dram_tensor("xT", [P, 4, ND, TCH], bf, kind="ExternalInput").ap()
    wqT = nc.dram_tensor("wqT", [P, HLOC, ND, P], bf, kind="ExternalInput").ap()
    kcT = nc.dram_tensor("kcT", [P, T], bf, kind="ExternalInput").ap()
    vp = nc.dram_tensor("vp", [P, NT, DK], bf, kind="ExternalInput").ap()
    woT = nc.dram_tensor("woT", [P, HLOC, DM], bf, kind="ExternalInput").ap()
    ct = nc.dram_tensor("ct", [P, T], bf, kind="ExternalInput").ap()
    st = nc.dram_tensor("st", [P, T], bf, kind="ExternalInput").ap()
    tri = nc.dram_tensor("tri", [P, P], bf, kind="ExternalInput").ap()
    out = nc.dram_tensor("out", [T, DM], bf, kind="ExternalOutput").ap()
    out_r = out.rearrange("(ti p) f -> p ti f", p=P)

    with tile.TileContext(nc) as tc:
        with (
            tc.tile_pool(name="res", bufs=1) as res,
            tc.tile_pool(name="work", bufs=6) as work,
            tc.tile_pool(name="probs", bufs=3) as probs,
            tc.tile_pool(name="accp", bufs=2) as accp,
            tc.tile_pool(name="small", bufs=4) as small,
            tc.tile_pool(name="bcast", bufs=2) as bcastp,
            tc.tile_pool(name="outp", bufs=3) as outp,
            tc.tile_pool(name="ps_sc", bufs=2, space="PSUM") as ps_sc,
            tc.tile_pool(name="ps_wo", bufs=2, space="PSUM") as ps_wo,
            tc.tile_pool(name="ps_gx", bufs=2, space="PSUM") as ps_gx,
        ):
            # ---- loads, in consumption order; x split in half-chunks so
            # the first projection matmuls start ~5.6us in ----
            wq_sb = res.tile([P, HLOC, ND, P], bf)
            nc.sync.dma_start(wq_sb[:, 0], wqT[:, 0])
            eps_sb = res.tile([P, 1], f32)
            nc.vector.memset(eps_sb, EPS)
            # all-ones stationary: ones128.T @ x = column sums broadcast to
            # every output partition — reduce+broadcast in one 213ns matmul
            ones_sb = res.tile([P, P], bf)
            nc.vector.memset(ones_sb, 1.0)
            x_sb = res.tile([P, 4, ND, TCH], bf)
            ct_sb = res.tile([P, T], bf)
            st_sb = res.tile([P, T], bf)
            kc_sb = res.tile([P, T], bf)
            vp_sb = res.tile([P, NT, DK], bf)
            tri_sb = res.tile([P, P], bf)
            wo_sb = res.tile([P, HLOC, DM], bf)
            cs0 = slice(0, TCH)
            nc.sync.dma_start(x_sb[:, 0, 0:4], xT[:, 0, 0:4])
            nc.sync.dma_start(x_sb[:, 0, 4:8], xT[:, 0, 4:8])
            nc.sync.dma_start(wq_sb[:, 1], wqT[:, 1])
            nc.sync.dma_start(x_sb[:, 0, 8:16], xT[:, 0, 8:16])
            nc.sync.dma_start(ct_sb[:, cs0], ct[:, cs0])
            nc.sync.dma_start(st_sb[:, cs0], st[:, cs0])
            nc.sync.dma_start(kc_sb, kcT)
            nc.sync.dma_start(vp_sb, vp)
            nc.sync.dma_start(tri_sb, tri)
            for tck in range(1, 4):
                csk = slice(tck * TCH, (tck + 1) * TCH)
                if tck == 3:
                    nc.sync.dma_start(wo_sb, woT)
                nc.sync.dma_start(x_sb[:, tck, 0:8], xT[:, tck, 0:8])
                nc.sync.dma_start(x_sb[:, tck, 8:16], xT[:, tck, 8:16])
                nc.sync.dma_start(ct_sb[:, csk], ct[:, csk])
                nc.sync.dma_start(st_sb[:, csk], st[:, csk])

            qr = [res.tile([P, T], bf, name=f"qr{h}") for h in range(HLOC)]
            qn = [res.tile([P, T], bf, name=f"qn{h}") for h in range(HLOC)]
            attT = [res.tile([P, T], bf, name=f"attT{h}") for h in range(HLOC)]

            # ---- phase B: transposed q projection + rope + rmsnorm ----
            def proj_chunk(tck, h):
                cs = slice(tck * TCH, (tck + 1) * TCH)
                ps = ps_wo.tile([P, TCH], f32, tag="pw")
                for dc in range(ND):
                    nc.tensor.matmul(
                        ps,
                        lhsT=wq_sb[:, h, dc, :],
                        rhs=x_sb[:, tck, dc, :],
                        start=(dc == 0),
                        stop=(dc == ND - 1),
                    )
                qsb = work.tile([P, TCH], bf, tag="qsb")
                nc.vector.tensor_copy(qsb, ps)
                # rope: qr = q*CT + swap64(q)*ST.  st_sb holds ST with
                # halves pre-swapped so each mul's two SB inputs share a
                # base partition (only the OUT is partition-shifted).
                u = work.tile([P, TCH], bf, tag="u")
                nc.vector.tensor_mul(
                    u[0:64, :], qsb[64:128, :], st_sb[64:128, cs])
                nc.vector.tensor_mul(
                    u[64:128, :], qsb[0:64, :], st_sb[0:64, cs])
                t1 = work.tile([P, TCH], bf, tag="t1")
                nc.vector.tensor_mul(t1, qsb, ct_sb[:, cs])
                nc.vector.tensor_add(qr[h][:, cs], t1, u)
                # rmsnorm: sumsq bcast via ones-matmul, then
                # rstd = 1/sqrt(ssq/DK + eps) via Sqrt (ACT) + fast recip
                sq = work.tile([P, TCH], bf, tag="sq")
                nc.vector.tensor_mul(sq, qsb, qsb)
                ssqb = ps_gx.tile([P, TCH], f32, tag="gx")
                nc.tensor.matmul(
                    ssqb, lhsT=ones_sb, rhs=sq, start=True, stop=True)
                srt = work.tile([P, TCH], f32, tag="srt")
                nc.scalar.activation(
                    out=srt, in_=ssqb, func=AF.Sqrt,
                    bias=eps_sb, scale=1.0 / DK)
                rstdb = work.tile([P, TCH], f32, tag="rstdb")
                nc.vector.reciprocal_approx_fast(rstdb, srt)
                nc.vector.tensor_mul(qn[h][:, cs], qr[h][:, cs], rstdb)

            # ---- phase C pieces ----
            # wo chains queue up as closures; attention drains one per
            # scores/attv pair so PE always has independent filler work
            filler = []

            def emit_filler(n=1):
                for _ in range(n):
                    if filler:
                        filler.pop(0)()

            def att_group(g, h, fill=False):
                gs = slice(g * GW * P, (g + 1) * GW * P)
                po = ps_gx.tile([P, GW * P], f32, tag="gx")
                acc = accp.tile([P, GW * P], bf, tag="acc")
                dnb = ps_gx.tile([P, GW * P], f32, tag="gx")
                nsi = GW * (g + 1)
                pbs = []

                def scores_pair(pi):
                    ps2 = ps_sc.tile([P, 2 * GW * P], f32, tag="sc")
                    los = []
                    for j in range(2):
                        si = pi * 2 + j
                        k = si - GW * g
                        lo = k * P if k > 0 else 0
                        los.append(lo)
                        nc.tensor.matmul(
                            ps2[:, j * 512 + lo:(j + 1) * 512],
                            lhsT=kc_sb[:, si * P:(si + 1) * P],
                            rhs=qn[h][:, g * 512 + lo:(g + 1) * 512],
                            start=True, stop=True,
                        )
                    pb = probs.tile([P, 2 * GW * P], bf, tag="pb")
                    nc.scalar.activation(
                        out=pb[:, los[0]:], in_=ps2[:, los[0]:],
                        func=AF.Exp)
                    for j in range(2):
                        si = pi * 2 + j
                        k = si - GW * g
                        if k >= 0:
                            dsl = slice(j * 512 + k * P,
                                        j * 512 + (k + 1) * P)
                            nc.gpsimd.tensor_mul(
                                pb[:, dsl], pb[:, dsl], tri_sb)
                    pbs.append(pb)

                def attv_pair(pi):
                    pb = pbs[pi]
                    for j in range(2):
                        si = pi * 2 + j
                        k = si - GW * g
                        lo = k * P if k > 0 else 0
                        if k < 0:
                            # full tile: denominator row rides on PE
                            # (ones-matmul accumulate into dnb)
                            nc.tensor.matmul(
                                dnb, lhsT=ones_sb,
                                rhs=pb[:, j * 512:(j + 1) * 512],
                                start=(si == 0), stop=False,
                            )
                        else:
                            # diagonal tile: accumulate on DVE in bf16
                            # (noise averages out in the reduction)
                            if k == 0:
                                nc.vector.tensor_copy(
                                    acc, pb[:, j * 512:(j + 1) * 512])
                            else:
                                nc.vector.tensor_add(
                                    acc[:, lo:512], acc[:, lo:512],
                                    pb[:, j * 512 + lo:(j + 1) * 512])
                        nc.tensor.matmul(
                            po[:, lo:512],
                            lhsT=vp_sb[:, si, :],
                            rhs=pb[:, j * 512 + lo:(j + 1) * 512],
                            start=(si == 0), stop=(si == nsi - 1),
                        )

                npairs = 2 * (g + 1)
                for pi in range(npairs):
                    scores_pair(pi)
                    if fill:
                        emit_filler()
                    if pi >= 1:
                        attv_pair(pi - 1)
                        if fill:
                            emit_filler()
                attv_pair(npairs - 1)
                if fill:
                    emit_filler()
                # fold the diagonal accumulator into dnb, closing the
                # PE accumulation group
                nc.tensor.matmul(
                    dnb, lhsT=ones_sb, rhs=acc,
                    start=(g == 0), stop=True)

                # reciprocal -> normalize-evacuate
                rcb = bcastp.tile([P, GW * P], f32, tag="rcb")
                nc.vector.reciprocal_approx_fast(rcb, dnb)
                nc.vector.tensor_mul(attT[h][:, gs], po, rcb)

            def wo_chain(ti, fc, evac_eng, pool=None):
                fs = slice(fc * 512, (fc + 1) * 512)
                pw = (pool or ps_wo).tile(
                    [P, 512], f32, tag="pw" if pool is None else "gx")
                nc.tensor.matmul(
                    pw,
                    lhsT=attT[0][:, ti * P:(ti + 1) * P],
                    rhs=wo_sb[:, 0, fs],
                    start=True, stop=False,
                )
                nc.tensor.matmul(
                    pw,
                    lhsT=attT[1][:, ti * P:(ti + 1) * P],
                    rhs=wo_sb[:, 1, fs],
                    start=False, stop=True,
                )
                osb = outp.tile([P, 512], bf, tag="osb")
                if evac_eng == 0:
                    nc.vector.tensor_copy(osb, pw)
                else:
                    nc.scalar.copy(osb, pw)
                nc.sync.dma_start(out_r[:, ti, fs], osb)

            def push_wo(tis, dve_frac=2, pools=False):
                # dve_frac of 3 evacs to DVE, rest to Scalar; pools=True
                # alternates PSUM pools (tail: gx ring is free then)
                for i, (ti, fc) in enumerate(
                        [(ti, fc) for ti in tis for fc in range(4)]):
                    eng = 0 if (i % 3) < dve_frac else 1
                    pl = ps_gx if (pools and i % 2) else None
                    filler.append(
                        lambda ti=ti, fc=fc, e=eng, p=pl: wo_chain(
                            ti, fc, e, p))

            # ---- emission schedule ----
            # g0's attention interleaves into the projection phase (PE
            # filler while x chunks stream in); the remaining groups run
            # largest-first so PE stays dense and HAM stays warm; wo
            # chains drain as filler once wo_sb has landed (~att g2).
            proj_chunk(0, 0)
            proj_chunk(0, 1)
            att_group(0, 0)
            att_group(0, 1)
            for tck in range(1, 4):
                proj_chunk(tck, 0)
                proj_chunk(tck, 1)
            push_wo([0, 1, 2, 3], dve_frac=1)
            att_group(3, 0, fill=True)
            att_group(3, 1, fill=True)
            push_wo([12, 13, 14, 15], dve_frac=1)
            att_group(2, 0, fill=True)
            att_group(2, 1, fill=True)
            push_wo([8, 9, 10, 11], dve_frac=2)
            att_group(1, 0, fill=True)
            att_group(1, 1, fill=True)
            push_wo([4, 5, 6, 7], dve_frac=2)
            emit_filler(len(filler))

    nc.compile()
    return nc


def _host_inputs(x, cached_k, cached_v, wq, wo, q_norm_w):
    """Build the 8 per-core input maps (host-side shard + fold + cast)."""
    xt = np.ascontiguousarray(x[0].T).astype(np.float32)          # (DM, T)
    # x_prep[p, tck, dc, t'] = x[tck*512+t', dc*128+p]
    x_prep = np.ascontiguousarray(
        xt.reshape(ND, P, 4, TCH).transpose(1, 2, 0, 3)).astype(_bf16)

    inv_freq = 1.0 / (ROPE_BASE ** (np.arange(0, DK, 2, dtype=np.float32) / DK))
    ang = np.arange(T, dtype=np.float32)[:, None] * inv_freq[None, :]  # (T,64)
    w = q_norm_w.astype(np.float32)
    cosT = np.cos(ang).T                                           # (64, T)
    sinT = np.sin(ang).T
    CT = np.concatenate([cosT, cosT], axis=0) * w[:, None]         # (128, T)
    ST = np.empty((DK, T), np.float32)
    ST[:64] = -w[64:, None] * sinT
    ST[64:] = w[:64, None] * sinT
    CTb = np.ascontiguousarray(CT).astype(_bf16)
    # halves pre-swapped: st_sb[p] = ST[(p+64) % 128]
    STb = np.ascontiguousarray(
        np.concatenate([ST[64:], ST[:64]], axis=0)).astype(_bf16)

    tri_m = (np.arange(P)[:, None] <= np.arange(P)[None, :]).astype(_bf16)

    in_maps = []
    for c in range(NCORES):
        # wq_prep[p, h, dc, m] = wq[(2c+h)*128+m, dc*128+p]
        wqs = wq[c * HLOC * DK:(c + 1) * HLOC * DK, :].astype(np.float32)
        wq_prep = np.ascontiguousarray(
            wqs.reshape(HLOC, P, ND, P).transpose(3, 0, 2, 1)).astype(_bf16)
        kcp = np.ascontiguousarray(
            cached_k[c].T / math.sqrt(DK)).astype(_bf16)           # (128, T)
        vpp = np.ascontiguousarray(
            cached_v[c].reshape(NT, P, DK).transpose(1, 0, 2)).astype(_bf16)
        # wo_prep[p, h, f] = wo[f, (2c+h)*128+p]
        wos = wo[:, c * HLOC * DK:(c + 1) * HLOC * DK].astype(np.float32)
        wo_prep = np.ascontiguousarray(
            wos.reshape(DM, HLOC, P).transpose(2, 1, 0)).astype(_bf16)
        in_maps.append({
            "xT": x_prep, "wqT": wq_prep, "kcT": kcp, "vp": vpp,
            "woT": wo_prep, "ct": CTb, "st": STb, "tri": tri_m,
        })
    return in_maps


_CACHED = {}


def _get_module():
    if "nc" not in _CACHED:
        _CACHED["nc"] = _build_module()
    return _CACHED["nc"]


def run(inputs, trace=False, **kw):
    """Compile (cached), run on 8 cores, return (output, BassKernelResults)."""
    from concourse import bass_utils

    nc = _get_module()
    in_maps = _host_inputs(
        np.asarray(inputs["x"], np.float32),
        np.asarray(inputs["cached_k"], np.float32),
        np.asarray(inputs["cached_v"], np.float32),
        np.asarray(inputs["wq"], np.float32),
        np.asarray(inputs["wo"], np.float32),
        np.asarray(inputs["q_norm_w"], np.float32),
    )
    res = bass_utils.run_bass_kernel_spmd(
        nc, in_maps, core_ids=list(range(NCORES)), trace=trace, **kw)
    full = np.zeros((T, DM), np.float32)
    for c in range(NCORES):
        full += res.results[c]["out"].astype(np.float32)
    return full.reshape(1, T, DM), res


def kernel(**inputs):
    full, _ = run(inputs)
    return full
